# revision 6
# baseline (speedup 1.0000x reference)
"""Gumbel Top-K soft-mask kernel for Trainium2 (8 NeuronCores, SPMD).

Algorithm (matches reference.py up to fp rounding):
    perturbed = logits + gumbel_noise            (temperature = 1)
    repeat k times:
        p = softmax(remaining)                   (row-wise)
        remaining -= p * 1e6
    mask = clip((perturbed - remaining) * 1e-6, 0, 1)

The final differencing recovers sum(p_i) without accumulating a separate
soft-mask tensor.  Per iteration the device does:
    e, s = Exp(remaining, bias=-b), row-sum      (ScalarE, one pass)
    r6   = 1e6 / s                               (VectorE, [128,1])
    remaining = remaining - e*r6                 (VectorE ln_bwd_dx, one pass)
    b   += ln(s)                                 (ScalarE [128,1] + VectorE [128,1])
b is a per-row running log-sum-exp offset: it keeps every Exp argument in
(-40, 0] so no row can underflow to s == 0 (which would produce NaN) and
1e6/s stays far from f32 overflow.

Sharding: batch rows 8192 split 1024-per-core across 8 cores; each row's
softmax is row-local so there is no cross-core communication.
"""

import numpy as np

import concourse.bacc as bacc
import concourse.mybir as mybir
import concourse.tile as tile
from concourse.bass_utils import run_bass_kernel_spmd

N_CORES = 8
BATCH = 8192
N = 4096
ROWS_PER_CORE = BATCH // N_CORES  # 1024
P = 128
N_TILES = ROWS_PER_CORE // P  # 8
PENALTY = 1.0e6
B0 = 25.0  # initial exp offset; > max(perturbed) for N(0,1)+Gumbel data

_CACHE: dict[int, object] = {}

F32 = mybir.dt.float32


def _build(k: int):
    nc = bacc.Bacc(
        "TRN2",
        target_bir_lowering=False,
        debug=False,
        enable_asserts=False,
        num_devices=N_CORES,
    )
    logits = nc.dram_tensor("logits", [ROWS_PER_CORE, N], F32, kind="ExternalInput").ap()
    noise = nc.dram_tensor("noise", [ROWS_PER_CORE, N], F32, kind="ExternalInput").ap()
    out = nc.dram_tensor("out", [ROWS_PER_CORE, N], F32, kind="ExternalOutput").ap()

    lg = logits.rearrange("(t p) n -> t p n", p=P)
    gn = noise.rearrange("(t p) n -> t p n", p=P)
    om = out.rearrange("(t p) n -> t p n", p=P)

    from contextlib import ExitStack

    with tile.TileContext(nc) as tc, ExitStack() as ctx:
        io_pool = ctx.enter_context(tc.tile_pool(name="io", bufs=2))
        pt_pool = ctx.enter_context(tc.tile_pool(name="pt", bufs=2))
        rt_pool = ctx.enter_context(tc.tile_pool(name="rt", bufs=2))
        e_pool = ctx.enter_context(tc.tile_pool(name="e", bufs=2))
        sm_pool = ctx.enter_context(tc.tile_pool(name="sm", bufs=6))
        c_pool = ctx.enter_context(tc.tile_pool(name="c", bufs=1))

        c1e6 = c_pool.tile([P, 1], F32)
        nc.vector.memset(c1e6[:], PENALTY)

        for t in range(N_TILES):
            lt = io_pool.tile([P, N], F32, tag="lt")
            nc.sync.dma_start(lt[:], lg[t])
            nt = io_pool.tile([P, N], F32, tag="nt")
            nc.sync.dma_start(nt[:], gn[t])

            pt = pt_pool.tile([P, N], F32)
            nc.gpsimd.tensor_add(pt[:], lt[:], nt[:])

            rt = rt_pool.tile([P, N], F32)
            e = e_pool.tile([P, N], F32)
            negb = sm_pool.tile([P, 1], F32, tag="negb")
            nc.vector.memset(negb[:], -B0)
            for i in range(k):
                src = pt if i == 0 else rt
                s = sm_pool.tile([P, 1], F32, tag="s")
                nc.scalar.activation(
                    e[:], src[:], mybir.ActivationFunctionType.Exp,
                    bias=negb[:], scale=1.0, accum_out=s[:],
                )
                r6 = sm_pool.tile([P, 1], F32, tag="r6")
                nc.vector.reciprocal(r6[:], s[:])
                nc.vector.tensor_scalar_mul(r6[:], r6[:], PENALTY)
                # rt = src - e*r6
                nc.vector.ln_bwd_dx(rt[:], src[:], e[:], r6[:], 0.0, 1.0)
                if i < k - 1:
                    # negb_{i+1} = negb_i - ln(s)
                    lns = sm_pool.tile([P, 1], F32, tag="lns")
                    nc.scalar.activation(
                        lns[:], s[:], mybir.ActivationFunctionType.Ln,
                        bias=0.0, scale=1.0,
                    )
                    nb_new = sm_pool.tile([P, 1], F32, tag="negb")
                    nc.vector.tensor_scalar(
                        nb_new[:], lns[:], -1.0, negb[:],
                        mybir.AluOpType.mult, mybir.AluOpType.add,
                    )
                    negb = nb_new

            # m = (pt - rt) * 1e-6, clipped to [0, 1]  (reuses e's slots; e is dead)
            m = e_pool.tile([P, N], F32, tag="e")
            nc.vector.ln_bwd_dx(m[:], pt[:], rt[:], 1.0, 0.0, 1.0 / PENALTY)
            nc.gpsimd.tensor_scalar(
                m[:], m[:], 1.0, 0.0, mybir.AluOpType.min, mybir.AluOpType.max
            )
            nc.sync.dma_start(om[t], m[:])

    nc.compile()
    return nc


def _get(k: int):
    if k not in _CACHE:
        _CACHE[k] = _build(k)
    return _CACHE[k]


def kernel(logits, gumbel_noise, k) -> np.ndarray:
    logits = np.ascontiguousarray(np.asarray(logits, dtype=np.float32))
    gumbel_noise = np.ascontiguousarray(np.asarray(gumbel_noise, dtype=np.float32))
    k = int(np.asarray(k))
    assert logits.shape == (BATCH, N) and gumbel_noise.shape == (BATCH, N)

    nc = _get(k)
    in_maps = [
        {
            "logits": logits[c * ROWS_PER_CORE : (c + 1) * ROWS_PER_CORE],
            "noise": gumbel_noise[c * ROWS_PER_CORE : (c + 1) * ROWS_PER_CORE],
        }
        for c in range(N_CORES)
    ]
    res = run_bass_kernel_spmd(nc, in_maps, core_ids=list(range(N_CORES)))
    return np.concatenate([res.results[c]["out"] for c in range(N_CORES)], axis=0)
